# revision 28
# baseline (speedup 1.0000x reference)
"""Trainium2 Bass kernel for nn_AttentionTanh (B=8, S=2048, F=1024, U=256).

Data-parallel over batch: each of the 8 NeuronCores computes the full
attention for one batch example. No collectives.

Per-core dataflow (all matmuls via TensorE, out = lhsT.T @ rhs):
  xT   [F, S]  (host-transposed input shard, F on partitions)
  QT   [u, s] = tanh(Wq.T @ x.T)  -> matmul(lhsT=Wq[f,u], rhs=xT[f,s])
  KT   [u, s] = tanh(Wk.T @ x.T)
  V    [s, u] = tanh(x @ Wv)      -> matmul(lhsT=xT[f,s], rhs=Wv[f,u])
                V gets a fused ones-column so the out-matmul also
                produces the softmax denominator (column U).
  eST  [t, q] = exp(scale * K q.T) -> matmul(lhsT=KT[u,t], rhs=QT[u,q])
                (tanh bounds scores to [-8, 8]; no max subtraction needed)
  out  [q, u] = (eST.T @ [V | 1]) row-normalized by its last column.
"""

import os
import sys

import numpy as np

for _p in ("/opt/trn_rl_repo", "/root/.axon_site/_ro/trn_rl_repo"):
    if os.path.isdir(_p) and _p not in sys.path:
        sys.path.append(_p)

import concourse.bass as bass
import concourse.mybir as mybir
import concourse.tile as tile
from concourse.bass_utils import run_bass_kernel_spmd

P = 128
B, S, F, U = 8, 2048, 1024, 256
FO, SO, UO = F // P, S // P, U // P  # 8, 16, 2
SB = 512                             # s-block width for DMA/projections
NSB = S // SB                        # 4
QB = 512                             # query-block width (free dim of eST)
NQB = S // QB                        # 4
SCALE = 1.0 / float(np.sqrt(F))      # 1/32
F32 = mybir.dt.float32

# Compute dtype for TensorE matmuls: "float32", "float32r", or "bfloat16".
CDT = "float32r"


def _split_matmul_waits(nc):
    """Walrus instruction structs have a single sem-wait slot (EventSemaphore
    has two). Peel excess waits onto NoOps (plain wait instructions on the
    same engine) inserted just before the overloaded instruction."""
    n = 0
    for bb in nc.m.functions[0].blocks:
        new_insts = []
        for inst in bb.instructions:
            cap = 2 if isinstance(inst, mybir.InstEventSemaphore) else 1
            if (
                inst.sync_info
                and inst.sync_info.on_wait
                and len(inst.sync_info.on_wait) > cap
            ):
                waits = list(inst.sync_info.on_wait)
                for w in waits[cap:]:
                    n += 1
                    nop = mybir.InstNoOp(name=f"I-xwait-{n}", ins=[], outs=[])
                    nop.engine = inst.engine
                    nop.sync_info = mybir.SyncInfo(on_wait=[w], on_update=[])
                    new_insts.append(nop)
                inst.sync_info.on_wait = waits[:cap]
            new_insts.append(inst)
        bb.instructions[:] = new_insts
    return n


def build_nc(cdt_name=CDT, split_waits=True):
    cdt = getattr(mybir.dt, cdt_name)
    store_dt = F32 if cdt == F32 else cdt
    # float32r shares the fp32 bit layout, so DRAM parameters can be declared
    # f32r directly and DMA'd without a rounding cast; bf16 still needs the
    # staged cast copy after DMA.
    in_dt = cdt if cdt == mybir.dt.float32r else F32
    needs_cast = store_dt != in_dt

    nc = bass.Bass()
    xT_d = nc.declare_dram_parameter("xT", [F, S], in_dt, isOutput=False)
    w_d = {
        k: nc.declare_dram_parameter(k, [F, U], in_dt, isOutput=False)
        for k in ("Wq", "Wk", "Wv")
    }
    # Output is produced transposed ([U, S]); the host wrapper transposes it
    # back. This lets the attention output matmuls run with the 512-wide exp
    # tile as the moving operand (streams long enough to hide LDWEIGHTS).
    out_d = nc.declare_dram_parameter("out", [U, S], F32, isOutput=True)

    TANH = mybir.ActivationFunctionType.Tanh
    EXP = mybir.ActivationFunctionType.Exp

    with tile.TileContext(nc) as tc:
        with (
            tc.tile_pool(name="wpool", bufs=1) as wpool,
            tc.tile_pool(name="xpool", bufs=1) as xpool,
            tc.tile_pool(name="xstage", bufs=2) as xstage,
            tc.tile_pool(name="qkv", bufs=1) as qkv,
            tc.tile_pool(
                name="exps", bufs=2 if store_dt == mybir.dt.bfloat16 else 1
            ) as exps,
            tc.tile_pool(name="smalls", bufs=1) as smalls,
            tc.tile_pool(name="recs", bufs=2) as recs,
            tc.tile_pool(name="evac", bufs=4) as evac,
            tc.tile_pool(name="ps_big", bufs=2, space="PSUM") as ps_big,
            tc.tile_pool(name="ps_v", bufs=2, space="PSUM") as ps_v,
            tc.tile_pool(name="ps_o", bufs=2, space="PSUM") as ps_o,
            tc.tile_pool(name="ps_d", bufs=1, space="PSUM") as ps_dp,
        ):
            # ---- weights: [F, U] -> [fi=128, fo=8, u=256] (+ cast) ----
            # bf16 staging slots are shared with the x loads (same tag).
            w_t = {}
            for k in ("Wq", "Wk", "Wv"):
                if needs_cast:
                    wstg = xstage.tile(
                        [P, FO, SB], in_dt, tag="stage", name=f"wstg_{k}"
                    )
                    wf = wstg[:, :, :U]
                else:
                    wf = wpool.tile([P, FO, U], in_dt, tag=f"{k}_in")
                nc.sync.dma_start(
                    wf[:], w_d[k][:].rearrange("(fo fi) u -> fi fo u", fi=P)
                )
                if needs_cast:
                    wc = wpool.tile([P, FO, U], store_dt, tag=f"{k}_c")
                    nc.vector.tensor_copy(wc[:], wf[:])
                    w_t[k] = wc
                else:
                    w_t[k] = wf

            # ---- xT: [F, S] -> [fi=128, fo=8, s=2048] in s-blocks ----
            xT = xpool.tile([P, FO, S], store_dt)
            xT_src = xT_d[:].rearrange("(fo fi) s -> fi fo s", fi=P)
            for sb in range(NSB):
                sl = slice(sb * SB, (sb + 1) * SB)
                if not needs_cast:
                    nc.sync.dma_start(xT[:, :, sl], xT_src[:, :, sl])
                else:
                    xs = xstage.tile([P, FO, SB], in_dt, tag="stage")
                    nc.sync.dma_start(xs[:], xT_src[:, :, sl])
                    nc.vector.tensor_copy(xT[:, :, sl], xs[:])

            # ---- projections (per s-block so PE starts as DMA lands) ----
            qT = qkv.tile([P, UO, S], store_dt, tag="qT")
            kT = qkv.tile([P, UO, S], store_dt, tag="kT")
            vv = qkv.tile([P, SO, U], store_dt, tag="vv")
            # [t, 2] ones — stationary operand of the softmax-denominator
            # matmuls (M=2 keeps fp32r's even-free-count rule satisfied), and
            # [2, 128] halves — stationary of the reciprocal partition-
            # broadcast matmul (0.5 * recip + 0.5 * recip = recip on all
            # partitions).
            ones2 = smalls.tile([P, 2], store_dt, tag="ones2")
            halves = smalls.tile([2, P], store_dt, tag="halves")
            if store_dt == F32:
                nc.vector.memset(ones2[:], 1.0)
                nc.vector.memset(halves[:], 0.5)
            else:
                # memset can't write f32r/bf16-typed rounded values directly;
                # memset f32 then round via tensor_copy.
                ones_f32 = smalls.tile([P, 2], F32, tag="ones_f32")
                nc.vector.memset(ones_f32[:], 1.0)
                nc.vector.tensor_copy(ones2[:], ones_f32[:])
                halves_f32 = smalls.tile([2, P], F32, tag="halves_f32")
                nc.vector.memset(halves_f32[:], 0.5)
                nc.vector.tensor_copy(halves[:], halves_f32[:])

            for sb in range(NSB):
                sl = slice(sb * SB, (sb + 1) * SB)
                for wname, dst in (("Wq", qT), ("Wk", kT)):
                    for uo in range(UO):
                        ps = ps_big.tile([P, SB], F32, tag="ps_big")
                        for fo in range(FO):
                            nc.tensor.matmul(
                                ps[:],
                                w_t[wname][:, fo, uo * P : (uo + 1) * P],
                                xT[:, fo, sl],
                                start=(fo == 0),
                                stop=(fo == FO - 1),
                            )
                        nc.scalar.activation(dst[:, uo, sl], ps[:], TANH)
                for so in range(sb * SB // P, (sb + 1) * SB // P):
                    ps = ps_v.tile([P, U], F32, tag="ps_v")
                    for fo in range(FO):
                        nc.tensor.matmul(
                            ps[:],
                            xT[:, fo, so * P : (so + 1) * P],
                            w_t["Wv"][:, fo, :],
                            start=(fo == 0),
                            stop=(fo == FO - 1),
                        )
                    nc.scalar.activation(vv[:, so, :], ps[:], TANH)

            # ---- attention per query block ----
            for qb in range(NQB):
                qsl = slice(qb * QB, (qb + 1) * QB)
                ex = exps.tile([P, SO, QB], store_dt, tag="ex")
                for to in range(SO):
                    ps = ps_big.tile([P, QB], F32, tag="ps_big")
                    for uo in range(UO):
                        nc.tensor.matmul(
                            ps[:],
                            kT[:, uo, to * P : (to + 1) * P],
                            qT[:, uo, qsl],
                            start=(uo == 0),
                            stop=(uo == UO - 1),
                        )
                    nc.scalar.activation(ex[:, to, :], ps[:], EXP, scale=SCALE)
                # softmax denominator: [2, QB] = ones.T @ eST
                ps_d = ps_dp.tile([2, QB], F32, tag="ps_d")
                for to in range(SO):
                    nc.tensor.matmul(
                        ps_d[:],
                        ones2[:],
                        ex[:, to, :],
                        start=(to == 0),
                        stop=(to == SO - 1),
                    )
                # reciprocal of both (identical) denominator rows, then
                # broadcast across partitions via PE: halves.T @ rec2.
                rec2 = recs.tile([2, QB], F32, tag="rec2")
                nc.vector.reciprocal(rec2[:], ps_d[:])
                if store_dt == F32:
                    rec2c = rec2
                else:
                    rec2c = recs.tile([2, QB], store_dt, tag="rec2c")
                    nc.vector.tensor_copy(rec2c[:], rec2[:])
                psb = ps_dp.tile([P, QB], F32, tag="ps_b")
                nc.tensor.matmul(psb[:], halves[:], rec2c[:], start=True, stop=True)
                recb = evac.tile([P, QB], F32, tag="recb")
                nc.vector.tensor_copy(recb[:], psb[:])
                # outT[u, q] = V.T @ eST, normalized by the denominator
                for uo in range(UO):
                    ps = ps_o.tile([P, QB], F32, tag="ps_o")
                    for to in range(SO):
                        nc.tensor.matmul(
                            ps[:],
                            vv[:, to, uo * P : (uo + 1) * P],
                            ex[:, to, :],
                            start=(to == 0),
                            stop=(to == SO - 1),
                        )
                    ot = evac.tile([P, QB], F32, tag="ot")
                    nc.vector.tensor_mul(ot[:], ps[:], recb[:])
                    nc.sync.dma_start(out_d[uo * P : (uo + 1) * P, qsl], ot[:])

    if split_waits:
        _split_matmul_waits(nc)
    return nc


_NC_CACHE = {}


def _get_nc(cdt_name=CDT):
    if cdt_name not in _NC_CACHE:
        _NC_CACHE[cdt_name] = build_nc(cdt_name)
    return _NC_CACHE[cdt_name]


def make_in_maps(x, Wq, Wk, Wv):
    Wq = np.ascontiguousarray(np.asarray(Wq, dtype=np.float32))
    Wk = np.ascontiguousarray(np.asarray(Wk, dtype=np.float32))
    Wv = np.ascontiguousarray(np.asarray(Wv, dtype=np.float32))
    return [
        {
            "xT": np.ascontiguousarray(np.asarray(x[b], dtype=np.float32).T),
            "Wq": Wq,
            "Wk": Wk,
            "Wv": Wv,
        }
        for b in range(B)
    ]


def kernel(x, Wq, Wk, Wv):
    nc = _get_nc()
    in_maps = make_in_maps(x, Wq, Wk, Wv)
    res = run_bass_kernel_spmd(nc, in_maps, core_ids=list(range(B)))
    # Device output is [U, S] per core; transpose back to [S, U].
    return np.stack(
        [np.asarray(res.results[i]["out"], dtype=np.float32).T for i in range(B)],
        axis=0,
    )


# revision 33
# speedup vs baseline: 1.1745x; 1.1745x over previous
"""Trainium2 Bass kernel for nn_AttentionTanh (B=8, S=2048, F=1024, U=256).

Data-parallel over batch: each of the 8 NeuronCores computes the full
attention for one batch example. No collectives.

Per-core dataflow (all matmuls via TensorE, out = lhsT.T @ rhs):
  xT   [F, S]  (host-transposed input shard, F on partitions)
  QT   [u, s] = tanh(Wq.T @ x.T)  -> matmul(lhsT=Wq[f,u], rhs=xT[f,s])
  KT   [u, s] = tanh(Wk.T @ x.T)
  V    [s, u] = tanh(x @ Wv)      -> matmul(lhsT=xT[f,s], rhs=Wv[f,u])
                V gets a fused ones-column so the out-matmul also
                produces the softmax denominator (column U).
  eST  [t, q] = exp(scale * K q.T) -> matmul(lhsT=KT[u,t], rhs=QT[u,q])
                (tanh bounds scores to [-8, 8]; no max subtraction needed)
  out  [q, u] = (eST.T @ [V | 1]) row-normalized by its last column.
"""

import os
import sys

import numpy as np

for _p in ("/opt/trn_rl_repo", "/root/.axon_site/_ro/trn_rl_repo"):
    if os.path.isdir(_p) and _p not in sys.path:
        sys.path.append(_p)

import concourse.bass as bass
import concourse.mybir as mybir
import concourse.tile as tile
from concourse.bass_utils import run_bass_kernel_spmd

P = 128
B, S, F, U = 8, 2048, 1024, 256
FO, SO, UO = F // P, S // P, U // P  # 8, 16, 2
SB = 512                             # s-block width for DMA/projections
NSB = S // SB                        # 4
QB = 512                             # query-block width (free dim of eST)
NQB = S // QB                        # 4
SCALE = 1.0 / float(np.sqrt(F))      # 1/32
VW = U + 2                           # V plus fused ones columns (even width
                                     # keeps fp32r's free-count rules happy)
F32 = mybir.dt.float32

# Compute dtype for TensorE matmuls: "float32", "float32r", or "bfloat16".
CDT = "float32r"


def _split_matmul_waits(nc):
    """Walrus instruction structs have a single sem-wait slot (EventSemaphore
    has two). Peel excess waits onto NoOps (plain wait instructions on the
    same engine) inserted just before the overloaded instruction."""
    n = 0
    for bb in nc.m.functions[0].blocks:
        new_insts = []
        for inst in bb.instructions:
            cap = 2 if isinstance(inst, mybir.InstEventSemaphore) else 1
            if (
                inst.sync_info
                and inst.sync_info.on_wait
                and len(inst.sync_info.on_wait) > cap
            ):
                waits = list(inst.sync_info.on_wait)
                for w in waits[cap:]:
                    n += 1
                    nop = mybir.InstNoOp(name=f"I-xwait-{n}", ins=[], outs=[])
                    nop.engine = inst.engine
                    nop.sync_info = mybir.SyncInfo(on_wait=[w], on_update=[])
                    new_insts.append(nop)
                inst.sync_info.on_wait = waits[:cap]
            new_insts.append(inst)
        bb.instructions[:] = new_insts
    return n


def build_nc(cdt_name=CDT, split_waits=True):
    cdt = getattr(mybir.dt, cdt_name)
    store_dt = F32 if cdt == F32 else cdt
    # float32r shares the fp32 bit layout, so DRAM parameters can be declared
    # f32r directly and DMA'd without a rounding cast; bf16 still needs the
    # staged cast copy after DMA.
    in_dt = cdt if cdt == mybir.dt.float32r else F32
    needs_cast = store_dt != in_dt

    nc = bass.Bass()
    xT_d = nc.declare_dram_parameter("xT", [F, S], in_dt, isOutput=False)
    w_d = {
        k: nc.declare_dram_parameter(k, [F, U], in_dt, isOutput=False)
        for k in ("Wq", "Wk", "Wv")
    }
    out_d = nc.declare_dram_parameter("out", [S, U], F32, isOutput=True)

    TANH = mybir.ActivationFunctionType.Tanh
    EXP = mybir.ActivationFunctionType.Exp

    with tile.TileContext(nc) as tc:
        with (
            tc.tile_pool(name="wpool", bufs=1) as wpool,
            tc.tile_pool(name="xpool", bufs=1) as xpool,
            tc.tile_pool(name="xstage", bufs=2) as xstage,
            tc.tile_pool(name="qkv", bufs=1) as qkv,
            tc.tile_pool(
                name="exps", bufs=2 if store_dt == mybir.dt.bfloat16 else 1
            ) as exps,
            tc.tile_pool(name="smalls", bufs=1) as smalls,
            tc.tile_pool(name="recs", bufs=2) as recs,
            tc.tile_pool(name="evac", bufs=4) as evac,
            tc.tile_pool(name="ps_big", bufs=2, space="PSUM") as ps_big,
            tc.tile_pool(name="ps_v", bufs=2, space="PSUM") as ps_v,
            tc.tile_pool(name="ps_o", bufs=2, space="PSUM") as ps_o,
            tc.tile_pool(name="ps_d", bufs=1, space="PSUM") as ps_dp,
        ):
            # ---- weights: [F, U] -> [fi=128, fo=8, u=256] (+ cast) ----
            # bf16 staging slots are shared with the x loads (same tag).
            w_t = {}
            for k in ("Wq", "Wk", "Wv"):
                if needs_cast:
                    wstg = xstage.tile(
                        [P, FO, SB], in_dt, tag="stage", name=f"wstg_{k}"
                    )
                    wf = wstg[:, :, :U]
                else:
                    wf = wpool.tile([P, FO, U], in_dt, tag=f"{k}_in")
                nc.sync.dma_start(
                    wf[:], w_d[k][:].rearrange("(fo fi) u -> fi fo u", fi=P)
                )
                if needs_cast:
                    wc = wpool.tile([P, FO, U], store_dt, tag=f"{k}_c")
                    nc.vector.tensor_copy(wc[:], wf[:])
                    w_t[k] = wc
                else:
                    w_t[k] = wf

            # ---- xT: [F, S] -> [fi=128, fo=8, s=2048] in s-blocks ----
            xT = xpool.tile([P, FO, S], store_dt)
            xT_src = xT_d[:].rearrange("(fo fi) s -> fi fo s", fi=P)
            for sb in range(NSB):
                sl = slice(sb * SB, (sb + 1) * SB)
                if not needs_cast:
                    nc.sync.dma_start(xT[:, :, sl], xT_src[:, :, sl])
                else:
                    xs = xstage.tile([P, FO, SB], in_dt, tag="stage")
                    nc.sync.dma_start(xs[:], xT_src[:, :, sl])
                    nc.vector.tensor_copy(xT[:, :, sl], xs[:])

            # ---- projections (per s-block so PE starts as DMA lands) ----
            qT = qkv.tile([P, UO, S], store_dt, tag="qT")
            kT = qkv.tile([P, UO, S], store_dt, tag="kT")
            # V gets two fused ones-columns: the out-matmul then also produces
            # the softmax denominator (cols U:U+2; two columns keep fp32r's
            # even-free-count rule satisfied).
            vv = qkv.tile([P, SO, VW], store_dt, tag="vv")
            if store_dt == F32:
                nc.vector.memset(vv[:, :, U:VW], 1.0)
            else:
                # memset can't write f32r/bf16-typed rounded values directly;
                # memset f32 then round via tensor_copy.
                ones_f32 = smalls.tile([P, SO, VW - U], F32, tag="ones_f32")
                nc.vector.memset(ones_f32[:], 1.0)
                nc.vector.tensor_copy(vv[:, :, U:VW], ones_f32[:])

            for sb in range(NSB):
                sl = slice(sb * SB, (sb + 1) * SB)
                for wname, dst in (("Wq", qT), ("Wk", kT)):
                    for uo in range(UO):
                        ps = ps_big.tile([P, SB], F32, tag="ps_big")
                        for fo in range(FO):
                            nc.tensor.matmul(
                                ps[:],
                                w_t[wname][:, fo, uo * P : (uo + 1) * P],
                                xT[:, fo, sl],
                                start=(fo == 0),
                                stop=(fo == FO - 1),
                            )
                        nc.scalar.activation(dst[:, uo, sl], ps[:], TANH)
                for so in range(sb * SB // P, (sb + 1) * SB // P):
                    ps = ps_v.tile([P, U], F32, tag="ps_v")
                    for fo in range(FO):
                        nc.tensor.matmul(
                            ps[:],
                            xT[:, fo, so * P : (so + 1) * P],
                            w_t["Wv"][:, fo, :],
                            start=(fo == 0),
                            stop=(fo == FO - 1),
                        )
                    nc.scalar.activation(vv[:, so, :U], ps[:], TANH)

            # ---- attention per query block ----
            for qb in range(NQB):
                qsl = slice(qb * QB, (qb + 1) * QB)
                ex = exps.tile([P, SO, QB], store_dt, tag="ex")
                for to in range(SO):
                    ps = ps_big.tile([P, QB], F32, tag="ps_big")
                    for uo in range(UO):
                        nc.tensor.matmul(
                            ps[:],
                            kT[:, uo, to * P : (to + 1) * P],
                            qT[:, uo, qsl],
                            start=(uo == 0),
                            stop=(uo == UO - 1),
                        )
                    nc.scalar.activation(ex[:, to, :], ps[:], EXP, scale=SCALE)
                for ss in range(QB // P):
                    s0 = qb * QB + ss * P
                    ps = ps_o.tile([P, VW], F32, tag="ps_o")
                    for to in range(SO):
                        nc.tensor.matmul(
                            ps[:],
                            ex[:, to, ss * P : (ss + 1) * P],
                            vv[:, to, :],
                            start=(to == 0),
                            stop=(to == SO - 1),
                        )
                    rec = recs.tile([P, 1], F32, tag="rec")
                    nc.vector.reciprocal(rec[:], ps[:, U : U + 1])
                    ot = evac.tile([P, U], F32, tag="ot")
                    nc.vector.tensor_scalar_mul(ot[:], ps[:, :U], rec[:])
                    nc.sync.dma_start(out_d[s0 : s0 + P, :], ot[:])

    if split_waits:
        _split_matmul_waits(nc)
    return nc


_NC_CACHE = {}


def _get_nc(cdt_name=CDT):
    if cdt_name not in _NC_CACHE:
        _NC_CACHE[cdt_name] = build_nc(cdt_name)
    return _NC_CACHE[cdt_name]


def make_in_maps(x, Wq, Wk, Wv):
    Wq = np.ascontiguousarray(np.asarray(Wq, dtype=np.float32))
    Wk = np.ascontiguousarray(np.asarray(Wk, dtype=np.float32))
    Wv = np.ascontiguousarray(np.asarray(Wv, dtype=np.float32))
    return [
        {
            "xT": np.ascontiguousarray(np.asarray(x[b], dtype=np.float32).T),
            "Wq": Wq,
            "Wk": Wk,
            "Wv": Wv,
        }
        for b in range(B)
    ]


def kernel(x, Wq, Wk, Wv):
    nc = _get_nc()
    in_maps = make_in_maps(x, Wq, Wk, Wv)
    res = run_bass_kernel_spmd(nc, in_maps, core_ids=list(range(B)))
    return np.stack(
        [np.asarray(res.results[i]["out"], dtype=np.float32) for i in range(B)],
        axis=0,
    )
